# revision 33
# baseline (speedup 1.0000x reference)
"""DLRM dot-interaction kernel for Trainium2 (8 NeuronCores, batch-sharded).

Per sample b: T = concat(dense[b], embs[b]) -> [27, 128]; Z = T @ T^T;
output = strict upper triangle of Z -> [351] fp32.

Per-core plan (2048 samples, 16 blocks of 128):
  - SWDGE cast-DMA loads 2 blocks at a time as [128 b, (f,d)] fp16.
  - PE transposes each [128 b, 128 d] feature slab into PSUM; DVE/ACT copy
    into f-major Tt [128 d, f*128+b] fp16 (contiguous copies).
  - Per-sample fp16 matmul: lhsT = rhs = strided AP [128 d, 32 f] (27 real
    features + 5 zero pads); out -> PSUM [32, 32] at partition 32*(b%4)
    (col-group tiling, 4 samples per PSUM partition dim).
  - ACT copies Z PSUM -> SBUF Zs [(g,m) part, (blk,q,n)] fp32, half-core span.
  - Triu extraction: SWDGE bounces Zs to DRAM scratch (full rows, big
    descriptors); then per (m, half) one HWDGE DRAM->DRAM gather DMA with
    1024 descriptors (spreads over ~8-16 DMA engines) packs z[m, m+1:27]
    runs into out[b, off_m:...].
"""

import numpy as np

B, NUM_EMBS, D = 16384, 26, 128
N_CORES = 8
BC = B // N_CORES  # 2048 samples per core
BLK = 128          # samples per block
NF = NUM_EMBS + 1  # 27 features
FP = 32            # feature pitch (27 + 5 pad)
NPAIR = NF * (NF - 1) // 2  # 351

_CACHE = {}


def build(bc=BC):
    import concourse.bacc as bacc
    import concourse.mybir as mybir
    from concourse.tile import TileContext
    from concourse.masks import make_identity

    fp16 = mybir.dt.float16
    fp32 = mybir.dt.float32

    nc = bacc.Bacc("TRN2", target_bir_lowering=False, debug=False)
    dense_t = nc.dram_tensor("dense", (bc, D), fp32, kind="ExternalInput")
    embs_t = nc.dram_tensor("embs", (bc, NUM_EMBS, D), fp32, kind="ExternalInput")
    out_t = nc.dram_tensor("out", (bc, NPAIR), fp32, kind="ExternalOutput")

    nblk = bc // BLK
    assert nblk % 2 == 0
    QBLK = min(4, nblk)  # blocks per quarter-group (Zs/scratch granularity)
    QG = 16              # 4-sample groups per PSUM Z tile

    # Input load plan: small groups first (fast pipeline start), 4-block
    # groups at steady state (fewer SWDGE generations).
    groups = []
    b = 0
    head = [1, 1, 2, 2, 2]
    while b < nblk:
        sz = min(head.pop(0) if head else 4, nblk - b)
        groups.append((b, sz))
        b += sz
    g_of = {}
    for gs, sz in groups:
        for i in range(sz):
            g_of[gs + i] = (gs, sz)

    with TileContext(nc) as tc:
        with (
            tc.tile_pool(name="consts", bufs=1) as consts,
            tc.tile_pool(name="xin", bufs=4) as xpool,
            tc.tile_pool(name="tt", bufs=4) as ttpool,
            tc.tile_pool(name="zsb", bufs=6) as zpool,
            tc.tile_pool(name="zb", bufs=3) as zbpool,
            tc.tile_pool(name="pk", bufs=3) as pkpool,
            tc.tile_pool(name="tp", bufs=4, space="PSUM") as tppool,
            tc.tile_pool(name="zp", bufs=4, space="PSUM") as zppool,
            tc.tile_pool(name="dscr", bufs=8, space="DRAM") as dpool,
        ):
            ident = consts.tile([128, 128], fp16)
            make_identity(nc, ident)

            dview = dense_t.ap()  # [bc, 128]
            eview = embs_t.ap().rearrange("b f d -> b (f d)")  # [bc, 3328]
            oview = out_t.ap()  # [bc, 351]

            X = None
            pend = []  # [(qtr, Zb)] deferred pack+out (emitted mid-next-quarter)
            zb5q = [None]  # current quarter's Zb 5-d view

            def pack_and_out(pqtr, Zb):
                # ---- pack triu (QBLK tiles wide per copy, DVE) ----
                Pk = pkpool.tile([128, QBLK * NPAIR], fp32, tag="Pk", name=f"Pk{pqtr}")
                zbp = Zb.rearrange(
                    "p (t c) -> p t c", t=QBLK
                )  # [128, t, 864+pad]
                pkp = Pk.rearrange("p (t c) -> p t c", t=QBLK)  # [128, t, 351]
                off = 0
                for m in range(NF - 1):
                    ln = NF - 1 - m
                    src = zbp[:, :, m * FP + m + 1 : m * FP + NF]
                    dst = pkp[:, :, off : off + ln]
                    # fp16 -> fp32 cast happens in the copy
                    if m % 3 == 2:
                        nc.scalar.copy(dst, src)
                    else:
                        nc.vector.tensor_copy(out=dst, in_=src)
                    off += ln

                # ---- output: per-g HWDGE DMA, 1404B runs ----
                b0q = pqtr * QBLK * BLK
                ovq = oview[b0q : b0q + QBLK * BLK].rearrange(
                    "(t qlo g) p -> g qlo t p", g=4, t=QBLK
                )  # [4, 32, t, 351]
                pk4 = pkp.rearrange("(g qlo) t c -> g qlo t c", g=4)
                for g in range(4):
                    # scalar only: out(q) follows pack(q) on the same queue
                    # with zero wait, and never head-of-line-blocks the sync
                    # queue's tail gathers behind a cross-engine pack wait
                    nc.scalar.dma_start(out=ovq[g], in_=pk4[g])

            for qtr in range(nblk // QBLK):
                Zb = zbpool.tile([128, QBLK * NF * FP], fp16, tag="Zb", name=f"Zb{qtr}")
                zb5q[0] = Zb.rearrange(
                    "(g qlo) (t m n) -> g qlo t m n", g=4, t=QBLK, n=FP
                )
                for pairi in range(max(1, QBLK // 2)):
                    npair = min(2, QBLK)
                    tts = []
                    # ---- phase 1: load + transpose for the block pair ----
                    for sub in range(npair):
                        blki = pairi * 2 + sub
                        blk = qtr * QBLK + blki
                        b0 = blk * BLK
                        gs, gsz = g_of[blk]
                        if blk == gs:
                            # SWDGE load casts fp32 -> fp16 at full rate
                            X = xpool.tile([BLK, gsz * NF * D], fp16, tag="X")
                            dsrc = dview[gs * BLK : (gs + gsz) * BLK].rearrange(
                                "(t b) d -> b t d", t=gsz
                            )  # [128, gsz, 128]
                            xd = X.rearrange("b (t c) -> b t c", t=gsz)
                            nc.gpsimd.dma_start(out=xd[:, :, 0:D], in_=dsrc)
                            esrc = eview[gs * BLK : (gs + gsz) * BLK].rearrange(
                                "(t b) c -> b t c", t=gsz
                            )  # [128, gsz, 3328]
                            nc.gpsimd.dma_start(out=xd[:, :, D:], in_=esrc)
                        xoff = (blk - gs) * NF * D

                        Tt = ttpool.tile([128, FP * D], fp16, tag="Tt")
                        # pad cols f=27..31 stay garbage: as weight/moving pads
                        # they only reach out-partitions/PSUM-cols >= 27 and
                        # Zs/Zb lanes the triu pack never reads
                        nchunk = (NF + 7) // 8
                        for ci in range(nchunk):
                            c0 = ci * 8
                            cf = min(8, NF - c0)
                            tp = tppool.tile([128, 8 * BLK], fp16, tag="tp")
                            for j in range(cf):
                                f = c0 + j
                                nc.tensor.transpose(
                                    tp[:, j * BLK : (j + 1) * BLK],
                                    X[:, xoff + f * D : xoff + (f + 1) * D],
                                    ident,
                                )
                            dst = Tt[:, c0 * BLK : (c0 + cf) * BLK]
                            src = tp[:, : cf * BLK]
                            if ci % 4 < 3:
                                nc.vector.tensor_copy(out=dst, in_=src)
                            else:
                                nc.scalar.copy(dst, src)
                        tts.append((blki, Tt))

                    # ---- phase 2: per-sample Gram matmuls (dense PE burst) --
                    for blki, Tt in tts:
                        Ttr = Tt.rearrange("d (f b) -> d b f", b=BLK)
                        nq = BLK // 4  # 32 groups of 4 samples
                        # First pair writes full 32-row strips so the PSUM
                        # pad partitions (27-31) are initialized once; after
                        # that, 27-column weights skip 5 LDWEIGHTS columns
                        # per sample and pads keep stale (unread) data.
                        mw = NF
                        Zs_t = zpool.tile([128, nq * FP], fp16, tag="Zs")
                        for qt in range(0, nq, QG):
                            zp = zppool.tile([128, QG * FP], fp32, tag="zp")
                            for q in range(QG):
                                for g in range(4):
                                    bloc = (qt + q) * 4 + g
                                    wop = Ttr[:, bloc, :mw]  # [128 d, 27 f]
                                    mop = Ttr[:, bloc, :NF]  # [128 d, 27 f]
                                    nc.tensor.matmul(
                                        zp[
                                            32 * g : 32 * g + mw,
                                            q * FP : q * FP + NF,
                                        ],
                                        wop,
                                        mop,
                                        start=True,
                                        stop=True,
                                        tile_position=(0, 32 * g),
                                    )
                            # copy Z PSUM -> SBUF block buffer, cast fp16
                            zcol0 = qt * FP
                            zdst = Zs_t[:, zcol0 : zcol0 + QG * FP]
                            if qt == 0:
                                nc.scalar.copy(zdst, zp[:, : QG * FP])
                            else:
                                nc.vector.tensor_copy(
                                    out=zdst, in_=zp[:, : QG * FP]
                                )
                        # ---- bounce this block's Z to DRAM scratch in raw
                        # partition order (one full-partition DMA, 2KB runs;
                        # per-block granularity keeps the chain pipelined)
                        scr_t = dpool.tile([128, nq * FP], fp16, tag="scr")
                        nc.gpsimd.dma_start(out=scr_t[:, :], in_=Zs_t[:, :])
                        # gather this block back sample-major immediately so
                        # the 64B-run reads spread over the whole quarter
                        sct = scr_t.rearrange(
                            "(g m) (q n) -> g q m n", g=4, n=FP
                        )
                        for g in range(4):
                            nc.sync.dma_start(
                                out=zb5q[0][g, :, blki], in_=sct[g, :, :NF, :]
                            )

                    if pairi == 0 and pend:
                        pack_and_out(*pend.pop(0))

                pend.append((qtr, Zb))

            while pend:
                pack_and_out(*pend.pop(0))

    nc.compile()
    return nc


def _get(bc=BC):
    if bc not in _CACHE:
        _CACHE[bc] = build(bc)
    return _CACHE[bc]


def kernel(dense: np.ndarray, embs: np.ndarray) -> np.ndarray:
    from concourse import bass_utils

    dense = np.ascontiguousarray(np.asarray(dense, dtype=np.float32))
    embs = np.ascontiguousarray(np.asarray(embs, dtype=np.float32))
    assert dense.shape == (B, D) and embs.shape == (B, NUM_EMBS, D)

    nc = _get()
    dsh = dense.reshape(N_CORES, BC, D)
    esh = embs.reshape(N_CORES, BC, NUM_EMBS, D)
    in_maps = [{"dense": dsh[i], "embs": esh[i]} for i in range(N_CORES)]
    res = bass_utils.run_bass_kernel_spmd(nc, in_maps, core_ids=list(range(N_CORES)))
    return np.concatenate([r["out"] for r in res.results], axis=0)

